# revision 31
# baseline (speedup 1.0000x reference)
"""EnhanceSelfAttention (B=2, S=2048, C=1024, H=16, D=64) on 8 trn2 cores.

Sharding: core c -> batch b = c // 4, head group g = c % 4 (heads 4g..4g+3).
Each core computes its 4 heads end-to-end plus a partial output projection
(rows of w_out for its heads); host sums the 4 partials per batch.

Schedule (v3), built around two measured facts from the v1 trace:
  - the attention inner loop is ACT-paced (exp [128,1024] = ~1147ns vs
    ~815ns of PE work per k-step), so ACT streams exp and nothing else
    until the post-exp tail;
  - input DMA is descriptor-rate-bound (~35ns/descriptor), so weights are
    host-packed into SBUF-layout megatiles (4KB rows), x streams in k-pair
    chunks (2KB rows) across queues, and cos/psin ship as 32 distinct rows
    replicated on-chip.
Other changes vs v1:
  - causal mask applied by DVE multiply (tri01) on the exp output
    instead of two PE matmuls per diagonal tile;
  - q_bias folded into the psum drain (tensor_scalar_add), not a K=1
    matmul; psum->SBUF drains on DVE; ACT takes the post-exp out-proj
    copies only;
  - attention chunks emit interleaved "filler" matmuls (later QKV
    chunks, V tiles, out-proj) between k-steps to fill PE slack during
    ACT-paced stretches; chunk order (0,0),(0,1),(1,1),(0,2),(1,2),
    (0,3),(1,3),(1,0) with quarter-granular RoPE for S-chunks 2/3;
  - softmax denominators ride the PV matmul as a 65th V column of ones;
    normalization = fast-reciprocal + gpsimd partition broadcast.
"""

import sys

if "/opt/trn_rl_repo" not in sys.path:
    sys.path.insert(0, "/opt/trn_rl_repo")

import numpy as np

import concourse.bacc as bacc
import concourse.bass as bass
import concourse.tile as tile
from concourse import mybir
from concourse.bass_utils import run_bass_kernel_spmd

B, S, C = 2, 2048, 1024
H, D = 16, 64
TEMP = 1e4
N_CORES = 8
HPC = 4            # heads per core
P = 128
NQC = S // 512     # 4 q-chunks of 512
KT = S // P        # 16 k-tiles
CKT = C // P       # 8 contraction tiles for projections

f32 = mybir.dt.float32
fp16 = mybir.dt.float16

_NC = None


def _build():
    nc = bacc.Bacc("TRN2", target_bir_lowering=False, debug=False)

    xT = nc.dram_tensor("xT", [C, S], fp16, kind="ExternalInput").ap()
    wqm = nc.dram_tensor("wqm", [P, CKT * 256], fp16, kind="ExternalInput").ap()
    wkm = nc.dram_tensor("wkm", [P, CKT * 256], fp16, kind="ExternalInput").ap()
    wvm = nc.dram_tensor("wvm", [P, CKT * 260], fp16, kind="ExternalInput").ap()
    wom = nc.dram_tensor("wom", [P, 2 * C], fp16, kind="ExternalInput").ap()
    qbt = nc.dram_tensor("qbt", [P, 2], f32, kind="ExternalInput").ap()
    vb = nc.dram_tensor("vb", [1, 260], f32, kind="ExternalInput").ap()
    cosT = nc.dram_tensor("cosT", [32, S], fp16, kind="ExternalInput").ap()
    psinT = nc.dram_tensor("psinT", [32, S], fp16, kind="ExternalInput").ap()
    tri01 = nc.dram_tensor("tri01", [P, P], fp16, kind="ExternalInput").ap()
    y = nc.dram_tensor("y", [S, C], fp16, kind="ExternalOutput").ap()

    with tile.TileContext(nc) as tc:
        _body(nc, tc, xT, wqm, wkm, wvm, wom, qbt, vb, cosT, psinT, tri01, y)
    nc.compile()
    return nc


def _body(nc, tc, xT, wqm, wkm, wvm, wom, qbt, vb, cosT, psinT, tri01, y):
    from contextlib import ExitStack

    with ExitStack() as ctx:
        consts = ctx.enter_context(tc.tile_pool(name="consts", bufs=1))

        xm = consts.tile([P, CKT * S], fp16, tag="xm", name="xm")
        wq_sb = consts.tile([P, CKT * 256], fp16, tag="wq", name="wq")
        wk_sb = consts.tile([P, CKT * 256], fp16, tag="wk", name="wk")
        wv_sb = consts.tile([P, CKT * 260], fp16, tag="wv", name="wv")
        wo_sb = consts.tile([P, 2 * C], fp16, tag="wo", name="wo")
        cos_sb = consts.tile([P, S], fp16, tag="cos", name="cos")
        psin_sb = consts.tile([P, S], fp16, tag="psin", name="psin")
        tri_sb = consts.tile([P, P], fp16, tag="tri", name="tri")
        qb_sb = consts.tile([P, 2], f32, tag="qb", name="qb")
        vb_sb = consts.tile([P, 260], f32, tag="vb", name="vb")
        warm = consts.tile([1, 2], f32, tag="warm", name="warm")
        wmm = consts.tile([P, 512], fp16, tag="wmm", name="wmm")

        qrot = [consts.tile([P, S], fp16, tag=f"qrot{t}", name=f"qrot{t}") for t in range(2)]
        krot = [consts.tile([P, S], fp16, tag=f"krot{t}", name=f"krot{t}") for t in range(2)]
        v_sb = [consts.tile([P, 260], fp16, tag=f"v{st}", name=f"v{st}") for st in range(KT)]
        attnT = [consts.tile([P, S], fp16, tag=f"attnT{p_}", name=f"attnT{p_}") for p_ in range(2)]

        # ---- input DMA: big-descriptor megatiles on sync/scalar/gpsimd ----
        xsrc = xT.rearrange("(k p) s -> p k s", p=P)
        xdst = xm[:].rearrange("p (k s) -> p k s", s=S)
        h0 = slice(0, 1024)
        h1 = slice(1024, 2048)
        # Priority ordering: each queue's transfers complete serially, so
        # late-needed bulk (x h1, wo) parks behind the critical stream.
        nc.sync.dma_start(xdst[:, 0:2, h0], xsrc[:, 0:2, h0])
        nc.sync.dma_start(xdst[:, 2:4, h0], xsrc[:, 2:4, h0])
        nc.sync.dma_start(xdst[:, 0:6, h1], xsrc[:, 0:6, h1])
        nc.sync.dma_start(xdst[:, 6:8, h1], xsrc[:, 6:8, h1])
        nc.sync.dma_start(wo_sb[:], wom[:])
        # weights on scalar (4KB descriptors); cos/psin ship as 32 rows
        nc.scalar.dma_start(wq_sb[:], wqm[:])
        nc.scalar.dma_start(wk_sb[:], wkm[:])
        nc.scalar.dma_start(cos_sb[0:32, :], cosT[:])
        nc.scalar.dma_start(psin_sb[0:32, :], psinT[:])
        # gpsimd: trailing x h0 k-tiles + small inputs + wv
        nc.gpsimd.dma_start(xdst[:, 4:6, h0], xsrc[:, 4:6, h0])
        nc.gpsimd.dma_start(xdst[:, 6:8, h0], xsrc[:, 6:8, h0])
        nc.gpsimd.dma_start(qb_sb[:], qbt[:])
        nc.gpsimd.dma_start(tri_sb[:], tri01[:])
        nc.gpsimd.dma_start(
            vb_sb[:],
            bass.AP(tensor=vb.tensor, offset=vb.offset, ap=[[0, P], [1, 260]]),
        )
        nc.gpsimd.dma_start(wv_sb[:], wvm[:])
        # replicate the 32 cos/psin rows to the other 3 partition blocks
        # (SBUF-SBUF, no HBM traffic); psin blocks 1,3 are negated
        for blk in range(1, 4):
            nc.gpsimd.dma_start(cos_sb[32 * blk:32 * blk + 32, :], cos_sb[0:32, :])
            nc.gpsimd.dma_start(psin_sb[32 * blk:32 * blk + 32, :], psin_sb[0:32, :])
        nc.vector.tensor_scalar_mul(psin_sb[32:64, :], psin_sb[32:64, :], -1.0)
        nc.vector.tensor_scalar_mul(psin_sb[96:128, :], psin_sb[96:128, :], -1.0)

        # ACT exp-table warm + warmup matmul data
        nc.vector.memset(warm[:], 0.0)
        nc.scalar.activation(warm[:], warm[:], mybir.ActivationFunctionType.Exp)
        nc.vector.memset(wmm[:], 0.0)

        # ---- pools (coexist; 8 PSUM banks total) ----
        accp = ctx.enter_context(tc.tile_pool(name="accp", bufs=4, space="PSUM"))
        scp = ctx.enter_context(tc.tile_pool(name="scps", bufs=2, space="PSUM"))
        bcosp = ctx.enter_context(tc.tile_pool(name="bcosp", bufs=6))
        swp = ctx.enter_context(tc.tile_pool(name="qkswp", bufs=4))
        exp_pool = ctx.enter_context(tc.tile_pool(name="expool", bufs=8))
        rcp_pool = ctx.enter_context(tc.tile_pool(name="rcppool", bufs=4))
        rbp = ctx.enter_context(tc.tile_pool(name="rbp", bufs=4))
        ob = ctx.enter_context(tc.tile_pool(name="obuf", bufs=6))

        # PE warm-up: release the HAM clock gate during the DMA window
        wps = accp.tile([P, 512], f32, tag="acc", name="acc")
        for _ in range(8):
            nc.tensor.matmul(wps[:], wmm[:, 0:P], wmm[:], start=True, stop=True)

        # ---------------- QKV projection + fused RoPE ---------------------
        base_t = {}   # (kind, t, half) -> [128, 1024] fp16 base tile

        def qk_chunk(kind, t, sc):
            """one [128, 512] psum accumulation for S-chunk sc; drain on DVE
            with q_bias folded in for q."""
            wsb = wq_sb if kind == "q" else wk_sb
            ps = accp.tile([P, 512], f32, tag="acc", name="acc")
            for k in range(CKT):
                nc.tensor.matmul(
                    ps[:],
                    wsb[:, k * 256 + t * P:k * 256 + (t + 1) * P],
                    xm[:, k * S + sc * 512:k * S + (sc + 1) * 512],
                    start=(k == 0), stop=(k == CKT - 1),
                )
            if (kind, t, sc // 2) not in base_t:
                base_t[(kind, t, sc // 2)] = bcosp.tile(
                    [P, 1024], fp16, tag="bcos", name="bcos")
            base = base_t[(kind, t, sc // 2)]
            dst = base[:, (sc % 2) * 512:(sc % 2) * 512 + 512]
            if kind == "q":
                nc.vector.tensor_scalar_add(dst, ps[:], qb_sb[:, t:t + 1])
            else:
                nc.vector.tensor_copy(dst, ps[:])

        def rope(kind, t, lo, w):
            """rotate base cols [lo, lo+w) (absolute S cols) into q/krot."""
            rot = (qrot if kind == "q" else krot)[t]
            span = slice(lo, lo + w)
            half = lo // 1024
            base = base_t[(kind, t, half)]
            bs = slice(lo - half * 1024, lo - half * 1024 + w)
            sw = swp.tile([P, 1024], fp16, tag="sw", name="sw")
            for blk in range(4):
                sb_, db = 32 * (blk ^ 1), 32 * blk
                nc.gpsimd.dma_start(sw[db:db + 32, bs], base[sb_:sb_ + 32, bs])
            nc.vector.tensor_mul(base[:, bs], base[:, bs], cos_sb[:, span])
            # psin = 32-block-swapped sin = -sin, so subtract: rot =
            # base*cos + swap(base)*sin = base*cos - swap(base)*psin
            nc.vector.tensor_mul(sw[:, bs], sw[:, bs], psin_sb[:, span])
            nc.vector.tensor_sub(rot[:, span], base[:, bs], sw[:, bs])

        def v_tile(st):
            ps = accp.tile([P, 260], f32, tag="acc", name="acc")
            for k in range(CKT):
                nc.tensor.matmul(
                    ps[:],
                    xm[:, k * S + st * P:k * S + (st + 1) * P],
                    wv_sb[:, k * 260:(k + 1) * 260],
                    start=(k == 0), stop=(k == CKT - 1),
                )
            nc.vector.tensor_add(v_sb[st][:], ps[:], vb_sb[:])

        # ---------------- attention with interleaved fillers --------------
        def attention(pair, n, fillers=()):
            fill = iter(fillers)

            def pump(cnt):
                for _ in range(cnt):
                    f = next(fill, None)
                    if f is None:
                        return
                    f()

            pv_ps = [accp.tile([65, 512], f32, tag="acc", name="acc")
                     for hh in range(2)]
            klast = 4 * n + 3

            def scores_step(k):
                diag = (k // 4 == n)
                off = P * (k % 4) if diag else 0
                sc = scp.tile([P, 2 * 512], f32, tag="sc", name="sc")
                for hh in range(2):
                    nc.tensor.matmul(
                        sc[:, hh * 512 + off:(hh + 1) * 512],
                        krot[pair][64 * hh:64 * hh + 64, k * P:(k + 1) * P],
                        qrot[pair][64 * hh:64 * hh + 64,
                                   n * 512 + off:(n + 1) * 512],
                        start=True, stop=True,
                        tile_position=(64 * hh, 0),
                    )
                ex = exp_pool.tile([P, 2 * 512], fp16, tag="ex", name="ex")
                if diag:
                    exr = ex[:].rearrange("p (h c) -> p h c", h=2)[:, :, off:]
                    scr = sc[:].rearrange("p (h c) -> p h c", h=2)[:, :, off:]
                    nc.scalar.activation(
                        exr, scr, mybir.ActivationFunctionType.Exp,
                        scale=0.125,
                    )
                    # causal mask: zero the upper triangle of the 128-wide
                    # diagonal block (DVE, not PE)
                    for hh in range(2):
                        blk = slice(hh * 512 + off, hh * 512 + off + P)
                        nc.vector.tensor_mul(ex[:, blk], ex[:, blk], tri_sb[:])
                else:
                    nc.scalar.activation(
                        ex[:], sc[:], mybir.ActivationFunctionType.Exp,
                        scale=0.125,
                    )
                return ex, off

            def pv_step(k, ex, off):
                for hh in range(2):
                    h = 2 * pair + hh
                    nc.tensor.matmul(
                        pv_ps[hh][:, off:512],
                        v_sb[k][:, 65 * h:65 * h + 65],
                        ex[:, hh * 512 + off:(hh + 1) * 512],
                        start=(k == 0), stop=(k == klast),
                    )

            # software pipeline: emit scores(k+1) before PV(k); pump
            # fillers into the PE slack of each ACT-paced step
            prev = None
            for k in range(klast + 1):
                cur = (k, *scores_step(k))
                pump(1)
                if prev is not None:
                    pv_step(*prev)
                    pump(1)
                prev = cur
            pv_step(*prev)
            span = slice(n * 512, (n + 1) * 512)
            dc = rcp_pool.tile([1, 1024], f32, tag="dc", name="dc")
            for hh in range(2):
                nc.vector.tensor_copy(dc[:, hh * 512:hh * 512 + 512],
                                      pv_ps[hh][64:65, :])
            rc = rcp_pool.tile([1, 1024], f32, tag="rc", name="rc")
            nc.vector.reciprocal_approx_fast(rc[:], dc[:])
            for hh in range(2):
                rb = rbp.tile([64, 512], f32, tag="rb", name="rb")
                nc.gpsimd.partition_broadcast(rb[:], rc[:, hh * 512:hh * 512 + 512])
                nc.vector.tensor_mul(
                    attnT[pair][64 * hh:64 * hh + 64, span],
                    pv_ps[hh][0:64, :],
                    rb[:],
                )
            # drain leftover fillers
            for f in fill:
                f()

        def out_proj(st, act_half=False):
            yo = ob.tile([P, C], fp16, tag="yo", name="yo")
            for c2 in range(2):
                po = accp.tile([P, 512], f32, tag="acc", name="acc")
                for kk in range(2):
                    nc.tensor.matmul(
                        po[:],
                        attnT[kk][:, st * P:(st + 1) * P],
                        wo_sb[:, kk * C + c2 * 512:kk * C + (c2 + 1) * 512],
                        start=(kk == 0), stop=(kk == 1),
                    )
                dst = yo[:, c2 * 512:(c2 + 1) * 512]
                if act_half and c2 == 1:
                    nc.scalar.activation(dst, po[:],
                                         mybir.ActivationFunctionType.Copy)
                else:
                    nc.vector.tensor_copy(dst, po[:])
            nc.sync.dma_start(y[st * P:(st + 1) * P, :], yo[:])

        # ---- filler generators: one closure ~= one matmul of PE work ----
        def qk_fillers(kind, t, sc):
            wsb = wq_sb if kind == "q" else wk_sb
            state = {}

            def mk(k):
                def emit():
                    if k == 0:
                        state["ps"] = accp.tile([P, 512], f32, tag="acc", name="acc")
                    nc.tensor.matmul(
                        state["ps"][:],
                        wsb[:, k * 256 + t * P:k * 256 + (t + 1) * P],
                        xm[:, k * S + sc * 512:k * S + (sc + 1) * 512],
                        start=(k == 0), stop=(k == CKT - 1),
                    )
                    if k == CKT - 1:
                        if (kind, t, sc // 2) not in base_t:
                            base_t[(kind, t, sc // 2)] = bcosp.tile(
                                [P, 1024], fp16, tag="bcos", name="bcos")
                        base = base_t[(kind, t, sc // 2)]
                        dst = base[:, (sc % 2) * 512:(sc % 2) * 512 + 512]
                        if kind == "q":
                            nc.vector.tensor_scalar_add(dst, state["ps"][:],
                                                        qb_sb[:, t:t + 1])
                        else:
                            nc.vector.tensor_copy(dst, state["ps"][:])
                return emit
            return [mk(k) for k in range(CKT)]

        def v_fillers(st):
            state = {}

            def mk(k):
                def emit():
                    if k == 0:
                        state["ps"] = accp.tile([P, 260], f32, tag="acc", name="acc")
                    nc.tensor.matmul(
                        state["ps"][:],
                        xm[:, k * S + st * P:k * S + (st + 1) * P],
                        wv_sb[:, k * 260:(k + 1) * 260],
                        start=(k == 0), stop=(k == CKT - 1),
                    )
                    if k == CKT - 1:
                        nc.vector.tensor_add(v_sb[st][:], state["ps"][:], vb_sb[:])
                return emit
            # 8 matmuls of 260 cols ~ 4 filler slots worth; group by 2
            fs = [mk(k) for k in range(CKT)]
            return [lambda a=fs[i], b=fs[i + 1]: (a(), b())
                    for i in range(0, CKT, 2)]

        def op_fillers(st, act_half=False):
            state = {}

            def mk(c2, kk):
                def emit():
                    if kk == 0:
                        state[c2] = accp.tile([P, 512], f32, tag="acc", name="acc")
                        if c2 == 0:
                            state["yo"] = ob.tile([P, C], fp16, tag="yo", name="yo")
                    nc.tensor.matmul(
                        state[c2][:],
                        attnT[kk][:, st * P:(st + 1) * P],
                        wo_sb[:, kk * C + c2 * 512:kk * C + (c2 + 1) * 512],
                        start=(kk == 0), stop=(kk == 1),
                    )
                    if kk == 1:
                        dst = state["yo"][:, c2 * 512:(c2 + 1) * 512]
                        if act_half and c2 == 1:
                            nc.scalar.activation(
                                dst, state[c2][:],
                                mybir.ActivationFunctionType.Copy)
                        else:
                            nc.vector.tensor_copy(dst, state[c2][:])
                        if c2 == 1:
                            nc.sync.dma_start(y[st * P:(st + 1) * P, :],
                                              state["yo"][:])
                return emit
            return [mk(c2, kk) for c2 in range(2) for kk in range(2)]

        def chain(*gens):
            out = []
            for g in gens:
                out.extend(g)
            return out

        # ---------------- emission order ----------------------------------
        # t0 q/k chunks for S-half 0, rope, so exp can start early
        qk_chunk("q", 0, 0)
        qk_chunk("k", 0, 0)
        qk_chunk("q", 0, 1)
        qk_chunk("k", 0, 1)
        rope("q", 0, 0, 1024)
        rope("k", 0, 0, 1024)
        qk_chunk("q", 1, 0)
        qk_chunk("k", 1, 0)
        qk_chunk("q", 1, 1)
        qk_chunk("k", 1, 1)
        rope("q", 1, 0, 1024)
        rope("k", 1, 0, 1024)
        for st in range(4):
            v_tile(st)

        attention(0, 0, chain(v_fillers(4), v_fillers(5)))
        attention(0, 1, chain(v_fillers(6), v_fillers(7),
                              qk_fillers("q", 0, 2), qk_fillers("k", 0, 2)))
        attention(1, 1, chain(qk_fillers("q", 1, 2), qk_fillers("k", 1, 2),
                              v_fillers(8), v_fillers(9)))
        rope("q", 0, 1024, 512)
        rope("k", 0, 1024, 512)
        rope("q", 1, 1024, 512)
        rope("k", 1, 1024, 512)
        attention(0, 2, chain(v_fillers(10), v_fillers(11),
                              qk_fillers("q", 0, 3), qk_fillers("k", 0, 3)))
        attention(1, 2, chain(qk_fillers("q", 1, 3), qk_fillers("k", 1, 3),
                              v_fillers(12), v_fillers(13)))
        rope("q", 0, 1536, 512)
        rope("k", 0, 1536, 512)
        rope("q", 1, 1536, 512)
        rope("k", 1, 1536, 512)
        attention(0, 3, chain(v_fillers(14), v_fillers(15),
                              op_fillers(4), op_fillers(5)))
        attention(1, 3, chain(op_fillers(6), op_fillers(7),
                              op_fillers(8), op_fillers(9),
                              op_fillers(10), op_fillers(11)))
        attention(1, 0, chain(op_fillers(12, act_half=True),
                              op_fillers(13, act_half=True),
                              op_fillers(14, act_half=True),
                              op_fillers(15, act_half=True)))
        for st in range(4):
            out_proj(st, act_half=True)


def _host_inputs(x, w_qkv, q_bias, v_bias, w_out):
    """Build the 8 per-core input maps (SBUF-layout megatiles)."""
    half = D // 2
    # pair-split column permutation within each head's 64 cols
    perm64 = np.empty(D, dtype=np.int64)
    perm64[:half] = 2 * np.arange(half)
    perm64[half:] = 2 * np.arange(half) + 1

    dim_t = (TEMP ** (np.arange(half, dtype=np.float32) / half)).astype(np.float32)
    ang = (np.arange(S, dtype=np.float32)[None, :] / dim_t[:, None]).astype(np.float32)
    cos32 = np.cos(ang).astype(np.float32)      # [32, S]
    # psin = 32-block-swapped sign pattern [+s,-s,+s,-s]; host sends +sin
    # rows, the kernel replicates and negates blocks 1 and 3 on-chip
    sin32 = np.sin(ang).astype(np.float32)

    r = np.arange(P)
    tri01 = (r[None, :] >= r[:, None]).astype(np.float16)

    Wq = w_qkv[:, 0:C]
    Wk = w_qkv[:, C:2 * C]
    Wv = w_qkv[:, 2 * C:3 * C]

    def mega(w):            # [C, X] -> [128, CKT*X] k-major
        X = w.shape[1]
        return np.ascontiguousarray(
            w.reshape(CKT, P, X).transpose(1, 0, 2).reshape(P, CKT * X))

    in_maps = []
    for core in range(N_CORES):
        b, g = core // HPC, core % HPC
        h0 = HPC * g
        cols = np.concatenate(
            [64 * h + perm64 for h in range(h0, h0 + HPC)]
        )                                        # permuted q/k cols, len 256
        vcols = np.arange(64 * h0, 64 * h0 + 256)
        wv260 = np.zeros((C, 260), dtype=np.float32)
        vb260 = np.zeros((1, 260), dtype=np.float32)
        wvc = Wv[:, vcols]
        vbc = v_bias[vcols]
        for hh_ in range(4):
            wv260[:, 65 * hh_:65 * hh_ + 64] = wvc[:, 64 * hh_:64 * hh_ + 64]
            vb260[0, 65 * hh_:65 * hh_ + 64] = vbc[64 * hh_:64 * hh_ + 64]
            vb260[0, 65 * hh_ + 64] = 1.0
        wo2 = w_out[64 * h0:64 * h0 + 256, :]    # [256, 1024]
        wom = np.ascontiguousarray(
            wo2.reshape(2, P, C).transpose(1, 0, 2).reshape(P, 2 * C))
        in_maps.append({
            "xT": np.ascontiguousarray(x[b].T).astype(np.float16),
            "wqm": mega(Wq[:, cols]).astype(np.float16),
            "wkm": mega(Wk[:, cols]).astype(np.float16),
            "wvm": mega(wv260).astype(np.float16),
            "wom": wom.astype(np.float16),
            "qbt": np.ascontiguousarray(
                q_bias[cols].reshape(2, P).T).astype(np.float32),
            "vb": vb260,
            "cosT": cos32.astype(np.float16), "psinT": sin32.astype(np.float16),
            "tri01": tri01,
        })
    return in_maps


def kernel(x, w_qkv, q_bias, v_bias, w_out, _trace=False):
    global _NC
    if _NC is None:
        _NC = _build()
    in_maps = _host_inputs(
        np.asarray(x, np.float32), np.asarray(w_qkv, np.float32),
        np.asarray(q_bias, np.float32), np.asarray(v_bias, np.float32),
        np.asarray(w_out, np.float32),
    )
    res = run_bass_kernel_spmd(_NC, in_maps, list(range(N_CORES)), trace=_trace)
    out = np.empty((B, S, C), dtype=np.float32)
    for b in range(B):
        acc = res.results[HPC * b]["y"].astype(np.float32)
        for g in range(1, HPC):
            acc += res.results[HPC * b + g]["y"].astype(np.float32)
        out[b] = acc
    if _trace:
        kernel.last_exec_time_ns = res.exec_time_ns
    return out


# revision 33
# speedup vs baseline: 1.0066x; 1.0066x over previous
"""EnhanceSelfAttention (B=2, S=2048, C=1024, H=16, D=64) on 8 trn2 cores.

Sharding: core c -> batch b = c // 4, head group g = c % 4 (heads 4g..4g+3).
Each core computes its 4 heads end-to-end plus a partial output projection
(rows of w_out for its heads); host sums the 4 partials per batch.

Schedule (v3), built around two measured facts from the v1 trace:
  - the attention inner loop is ACT-paced (exp [128,1024] = ~1147ns vs
    ~815ns of PE work per k-step), so ACT streams exp and nothing else
    until the post-exp tail;
  - input DMA is descriptor-rate-bound (~35ns/descriptor), so weights are
    host-packed into SBUF-layout megatiles (4KB rows), x streams in k-pair
    chunks (2KB rows) across queues, and cos/psin ship as 32 distinct rows
    replicated on-chip.
Other changes vs v1:
  - causal mask applied by DVE multiply (tri01) on the exp output
    instead of two PE matmuls per diagonal tile;
  - q_bias folded into the psum drain (tensor_scalar_add), not a K=1
    matmul; psum->SBUF drains on DVE; ACT takes the post-exp out-proj
    copies only;
  - attention chunks emit interleaved "filler" matmuls (later QKV
    chunks, V tiles, out-proj) between k-steps to fill PE slack during
    ACT-paced stretches; chunk order (0,0),(0,1),(1,1),(0,2),(1,2),
    (0,3),(1,3),(1,0) with quarter-granular RoPE for S-chunks 2/3;
  - softmax denominators ride the PV matmul as a 65th V column of ones;
    normalization = fast-reciprocal + gpsimd partition broadcast.
"""

import sys

if "/opt/trn_rl_repo" not in sys.path:
    sys.path.insert(0, "/opt/trn_rl_repo")

import numpy as np

import concourse.bacc as bacc
import concourse.bass as bass
import concourse.tile as tile
from concourse import mybir
from concourse.bass_utils import run_bass_kernel_spmd

B, S, C = 2, 2048, 1024
H, D = 16, 64
TEMP = 1e4
N_CORES = 8
HPC = 4            # heads per core
P = 128
NQC = S // 512     # 4 q-chunks of 512
KT = S // P        # 16 k-tiles
CKT = C // P       # 8 contraction tiles for projections

f32 = mybir.dt.float32
fp16 = mybir.dt.float16

_NC = None


def _build():
    nc = bacc.Bacc("TRN2", target_bir_lowering=False, debug=False)

    xT = nc.dram_tensor("xT", [C, S], fp16, kind="ExternalInput").ap()
    wqm = nc.dram_tensor("wqm", [P, CKT * 256], fp16, kind="ExternalInput").ap()
    wkm = nc.dram_tensor("wkm", [P, CKT * 256], fp16, kind="ExternalInput").ap()
    wvm = nc.dram_tensor("wvm", [P, CKT * 260], fp16, kind="ExternalInput").ap()
    wom = nc.dram_tensor("wom", [P, 2 * C], fp16, kind="ExternalInput").ap()
    qbt = nc.dram_tensor("qbt", [P, 2], f32, kind="ExternalInput").ap()
    vb = nc.dram_tensor("vb", [1, 260], f32, kind="ExternalInput").ap()
    cosT = nc.dram_tensor("cosT", [32, S], fp16, kind="ExternalInput").ap()
    psinT = nc.dram_tensor("psinT", [32, S], fp16, kind="ExternalInput").ap()
    tri01 = nc.dram_tensor("tri01", [P, P], fp16, kind="ExternalInput").ap()
    y = nc.dram_tensor("y", [S, C], fp16, kind="ExternalOutput").ap()

    with tile.TileContext(nc) as tc:
        _body(nc, tc, xT, wqm, wkm, wvm, wom, qbt, vb, cosT, psinT, tri01, y)
    nc.compile()
    return nc


def _body(nc, tc, xT, wqm, wkm, wvm, wom, qbt, vb, cosT, psinT, tri01, y):
    from contextlib import ExitStack

    with ExitStack() as ctx:
        consts = ctx.enter_context(tc.tile_pool(name="consts", bufs=1))

        xm = consts.tile([P, CKT * S], fp16, tag="xm", name="xm")
        wq_sb = consts.tile([P, CKT * 256], fp16, tag="wq", name="wq")
        wk_sb = consts.tile([P, CKT * 256], fp16, tag="wk", name="wk")
        wv_sb = consts.tile([P, CKT * 260], fp16, tag="wv", name="wv")
        wo_sb = consts.tile([P, 2 * C], fp16, tag="wo", name="wo")
        cos_sb = consts.tile([P, S], fp16, tag="cos", name="cos")
        psin_sb = consts.tile([P, S], fp16, tag="psin", name="psin")
        tri_sb = consts.tile([P, P], fp16, tag="tri", name="tri")
        qb_sb = consts.tile([P, 2], f32, tag="qb", name="qb")
        vb_sb = consts.tile([P, 260], f32, tag="vb", name="vb")
        warm = consts.tile([1, 2], f32, tag="warm", name="warm")
        wmm = consts.tile([P, 512], fp16, tag="wmm", name="wmm")

        qrot = [consts.tile([P, S], fp16, tag=f"qrot{t}", name=f"qrot{t}") for t in range(2)]
        krot = [consts.tile([P, S], fp16, tag=f"krot{t}", name=f"krot{t}") for t in range(2)]
        v_sb = [consts.tile([P, 260], fp16, tag=f"v{st}", name=f"v{st}") for st in range(KT)]
        attnT = [consts.tile([P, S], fp16, tag=f"attnT{p_}", name=f"attnT{p_}") for p_ in range(2)]

        # ---- input DMA: big-descriptor megatiles on sync/scalar/gpsimd ----
        xsrc = xT.rearrange("(k p) s -> p k s", p=P)
        xdst = xm[:].rearrange("p (k s) -> p k s", s=S)
        h0 = slice(0, 1024)
        h1 = slice(1024, 2048)
        # Priority ordering: each queue's transfers complete serially, so
        # late-needed bulk (x h1, wo) parks behind the critical stream.
        nc.sync.dma_start(xdst[:, 0:2, h0], xsrc[:, 0:2, h0])
        nc.sync.dma_start(xdst[:, 2:4, h0], xsrc[:, 2:4, h0])
        nc.sync.dma_start(xdst[:, 0:6, h1], xsrc[:, 0:6, h1])
        nc.sync.dma_start(xdst[:, 6:8, h1], xsrc[:, 6:8, h1])
        nc.sync.dma_start(wo_sb[:], wom[:])
        # weights on scalar (4KB descriptors); cos/psin ship as 32 rows
        nc.scalar.dma_start(wq_sb[:], wqm[:])
        nc.scalar.dma_start(wk_sb[:], wkm[:])
        nc.scalar.dma_start(cos_sb[0:32, :], cosT[:])
        nc.scalar.dma_start(psin_sb[0:32, :], psinT[:])
        # gpsimd: trailing x h0 k-tiles + small inputs + wv
        nc.gpsimd.dma_start(xdst[:, 4:6, h0], xsrc[:, 4:6, h0])
        nc.gpsimd.dma_start(xdst[:, 6:8, h0], xsrc[:, 6:8, h0])
        nc.gpsimd.dma_start(qb_sb[:], qbt[:])
        nc.gpsimd.dma_start(tri_sb[:], tri01[:])
        nc.gpsimd.dma_start(
            vb_sb[:],
            bass.AP(tensor=vb.tensor, offset=vb.offset, ap=[[0, P], [1, 260]]),
        )
        nc.gpsimd.dma_start(wv_sb[:], wvm[:])
        # replicate the 32 cos/psin rows to the other 3 partition blocks
        # (SBUF-SBUF, no HBM traffic); psin blocks 1,3 are negated
        for blk in range(1, 4):
            nc.gpsimd.dma_start(cos_sb[32 * blk:32 * blk + 32, :], cos_sb[0:32, :])
            nc.gpsimd.dma_start(psin_sb[32 * blk:32 * blk + 32, :], psin_sb[0:32, :])
        nc.vector.tensor_scalar_mul(psin_sb[32:64, :], psin_sb[32:64, :], -1.0)
        nc.vector.tensor_scalar_mul(psin_sb[96:128, :], psin_sb[96:128, :], -1.0)

        # ACT exp-table warm + warmup matmul data
        nc.vector.memset(warm[:], 0.0)
        nc.scalar.activation(warm[:], warm[:], mybir.ActivationFunctionType.Exp)
        nc.vector.memset(wmm[:], 0.0)

        # ---- pools (coexist; 8 PSUM banks total) ----
        accp = ctx.enter_context(tc.tile_pool(name="accp", bufs=4, space="PSUM"))
        scp = ctx.enter_context(tc.tile_pool(name="scps", bufs=2, space="PSUM"))
        bcosp = ctx.enter_context(tc.tile_pool(name="bcosp", bufs=4))
        swp = ctx.enter_context(tc.tile_pool(name="qkswp", bufs=3))
        exp_pool = ctx.enter_context(tc.tile_pool(name="expool", bufs=8))
        rcp_pool = ctx.enter_context(tc.tile_pool(name="rcppool", bufs=4))
        rbp = ctx.enter_context(tc.tile_pool(name="rbp", bufs=4))
        ob = ctx.enter_context(tc.tile_pool(name="obuf", bufs=6))

        # PE warm-up: release the HAM clock gate during the DMA window
        wps = accp.tile([P, 512], f32, tag="acc", name="acc")
        for _ in range(8):
            nc.tensor.matmul(wps[:], wmm[:, 0:P], wmm[:], start=True, stop=True)

        # ---------------- QKV projection + fused RoPE ---------------------
        base_t = {}   # (kind, t, half) -> [128, 1024] fp16 base tile

        def qk_chunk(kind, t, sc):
            """one [128, 512] psum accumulation for S-chunk sc; drain on DVE
            with q_bias folded in for q."""
            wsb = wq_sb if kind == "q" else wk_sb
            ps = accp.tile([P, 512], f32, tag="acc", name="acc")
            for k in range(CKT):
                nc.tensor.matmul(
                    ps[:],
                    wsb[:, k * 256 + t * P:k * 256 + (t + 1) * P],
                    xm[:, k * S + sc * 512:k * S + (sc + 1) * 512],
                    start=(k == 0), stop=(k == CKT - 1),
                )
            if (kind, t, sc // 2) not in base_t:
                base_t[(kind, t, sc // 2)] = bcosp.tile(
                    [P, 1024], fp16, tag="bcos", name="bcos")
            base = base_t[(kind, t, sc // 2)]
            dst = base[:, (sc % 2) * 512:(sc % 2) * 512 + 512]
            if kind == "q":
                nc.vector.tensor_scalar_add(dst, ps[:], qb_sb[:, t:t + 1])
            else:
                nc.vector.tensor_copy(dst, ps[:])

        def rope(kind, t, lo, w):
            """rotate base cols [lo, lo+w) (absolute S cols) into q/krot."""
            rot = (qrot if kind == "q" else krot)[t]
            span = slice(lo, lo + w)
            half = lo // 1024
            base = base_t[(kind, t, half)]
            bs = slice(lo - half * 1024, lo - half * 1024 + w)
            sw = swp.tile([P, 1024], fp16, tag="sw", name="sw")
            for blk in range(4):
                sb_, db = 32 * (blk ^ 1), 32 * blk
                nc.gpsimd.dma_start(sw[db:db + 32, bs], base[sb_:sb_ + 32, bs])
            nc.vector.tensor_mul(base[:, bs], base[:, bs], cos_sb[:, span])
            # psin = 32-block-swapped sin = -sin, so subtract: rot =
            # base*cos + swap(base)*sin = base*cos - swap(base)*psin
            nc.vector.tensor_mul(sw[:, bs], sw[:, bs], psin_sb[:, span])
            nc.vector.tensor_sub(rot[:, span], base[:, bs], sw[:, bs])

        def v_tile(st):
            ps = accp.tile([P, 260], f32, tag="acc", name="acc")
            for k in range(CKT):
                nc.tensor.matmul(
                    ps[:],
                    xm[:, k * S + st * P:k * S + (st + 1) * P],
                    wv_sb[:, k * 260:(k + 1) * 260],
                    start=(k == 0), stop=(k == CKT - 1),
                )
            nc.vector.tensor_add(v_sb[st][:], ps[:], vb_sb[:])

        # ---------------- attention with interleaved fillers --------------
        def attention(pair, n, fillers=()):
            fill = iter(fillers)

            def pump(cnt):
                for _ in range(cnt):
                    f = next(fill, None)
                    if f is None:
                        return
                    f()

            pv_ps = [accp.tile([65, 512], f32, tag="acc", name="acc")
                     for hh in range(2)]
            klast = 4 * n + 3

            def scores_step(k):
                diag = (k // 4 == n)
                off = P * (k % 4) if diag else 0
                sc = scp.tile([P, 2 * 512], f32, tag="sc", name="sc")
                for hh in range(2):
                    nc.tensor.matmul(
                        sc[:, hh * 512 + off:(hh + 1) * 512],
                        krot[pair][64 * hh:64 * hh + 64, k * P:(k + 1) * P],
                        qrot[pair][64 * hh:64 * hh + 64,
                                   n * 512 + off:(n + 1) * 512],
                        start=True, stop=True,
                        tile_position=(64 * hh, 0),
                    )
                ex = exp_pool.tile([P, 2 * 512], fp16, tag="ex", name="ex")
                if diag:
                    exr = ex[:].rearrange("p (h c) -> p h c", h=2)[:, :, off:]
                    scr = sc[:].rearrange("p (h c) -> p h c", h=2)[:, :, off:]
                    nc.scalar.activation(
                        exr, scr, mybir.ActivationFunctionType.Exp,
                        scale=0.125,
                    )
                    # causal mask: zero the upper triangle of the 128-wide
                    # diagonal block (DVE, not PE)
                    for hh in range(2):
                        blk = slice(hh * 512 + off, hh * 512 + off + P)
                        nc.vector.tensor_mul(ex[:, blk], ex[:, blk], tri_sb[:])
                else:
                    nc.scalar.activation(
                        ex[:], sc[:], mybir.ActivationFunctionType.Exp,
                        scale=0.125,
                    )
                return ex, off

            def pv_step(k, ex, off):
                for hh in range(2):
                    h = 2 * pair + hh
                    nc.tensor.matmul(
                        pv_ps[hh][:, off:512],
                        v_sb[k][:, 65 * h:65 * h + 65],
                        ex[:, hh * 512 + off:(hh + 1) * 512],
                        start=(k == 0), stop=(k == klast),
                    )

            # software pipeline: emit scores(k+1) before PV(k); pump
            # fillers into the PE slack of each ACT-paced step
            prev = None
            for k in range(klast + 1):
                cur = (k, *scores_step(k))
                pump(1)
                if prev is not None:
                    pv_step(*prev)
                    pump(1)
                prev = cur
            pv_step(*prev)
            span = slice(n * 512, (n + 1) * 512)
            dc = rcp_pool.tile([1, 1024], f32, tag="dc", name="dc")
            for hh in range(2):
                nc.vector.tensor_copy(dc[:, hh * 512:hh * 512 + 512],
                                      pv_ps[hh][64:65, :])
            rc = rcp_pool.tile([1, 1024], f32, tag="rc", name="rc")
            nc.vector.reciprocal_approx_fast(rc[:], dc[:])
            for hh in range(2):
                rb = rbp.tile([64, 512], f32, tag="rb", name="rb")
                nc.gpsimd.partition_broadcast(rb[:], rc[:, hh * 512:hh * 512 + 512])
                nc.vector.tensor_mul(
                    attnT[pair][64 * hh:64 * hh + 64, span],
                    pv_ps[hh][0:64, :],
                    rb[:],
                )
            # drain leftover fillers
            for f in fill:
                f()

        def out_proj(st, act_half=False):
            yo = ob.tile([P, C], fp16, tag="yo", name="yo")
            for c2 in range(2):
                po = accp.tile([P, 512], f32, tag="acc", name="acc")
                for kk in range(2):
                    nc.tensor.matmul(
                        po[:],
                        attnT[kk][:, st * P:(st + 1) * P],
                        wo_sb[:, kk * C + c2 * 512:kk * C + (c2 + 1) * 512],
                        start=(kk == 0), stop=(kk == 1),
                    )
                dst = yo[:, c2 * 512:(c2 + 1) * 512]
                if act_half and c2 == 1:
                    nc.scalar.activation(dst, po[:],
                                         mybir.ActivationFunctionType.Copy)
                else:
                    nc.vector.tensor_copy(dst, po[:])
            nc.sync.dma_start(y[st * P:(st + 1) * P, :], yo[:])

        # ---- filler generators: one closure ~= one matmul of PE work ----
        def qk_fillers(kind, t, sc):
            wsb = wq_sb if kind == "q" else wk_sb
            state = {}

            def mk(k):
                def emit():
                    if k == 0:
                        state["ps"] = accp.tile([P, 512], f32, tag="acc", name="acc")
                    nc.tensor.matmul(
                        state["ps"][:],
                        wsb[:, k * 256 + t * P:k * 256 + (t + 1) * P],
                        xm[:, k * S + sc * 512:k * S + (sc + 1) * 512],
                        start=(k == 0), stop=(k == CKT - 1),
                    )
                    if k == CKT - 1:
                        if (kind, t, sc // 2) not in base_t:
                            base_t[(kind, t, sc // 2)] = bcosp.tile(
                                [P, 1024], fp16, tag="bcos", name="bcos")
                        base = base_t[(kind, t, sc // 2)]
                        dst = base[:, (sc % 2) * 512:(sc % 2) * 512 + 512]
                        if kind == "q":
                            nc.vector.tensor_scalar_add(dst, state["ps"][:],
                                                        qb_sb[:, t:t + 1])
                        else:
                            nc.vector.tensor_copy(dst, state["ps"][:])
                return emit
            return [mk(k) for k in range(CKT)]

        def v_fillers(st):
            state = {}

            def mk(k):
                def emit():
                    if k == 0:
                        state["ps"] = accp.tile([P, 260], f32, tag="acc", name="acc")
                    nc.tensor.matmul(
                        state["ps"][:],
                        xm[:, k * S + st * P:k * S + (st + 1) * P],
                        wv_sb[:, k * 260:(k + 1) * 260],
                        start=(k == 0), stop=(k == CKT - 1),
                    )
                    if k == CKT - 1:
                        nc.vector.tensor_add(v_sb[st][:], state["ps"][:], vb_sb[:])
                return emit
            # 8 matmuls of 260 cols ~ 4 filler slots worth; group by 2
            fs = [mk(k) for k in range(CKT)]
            return [lambda a=fs[i], b=fs[i + 1]: (a(), b())
                    for i in range(0, CKT, 2)]

        def op_fillers(st, act_half=False):
            state = {}

            def mk(c2, kk):
                def emit():
                    if kk == 0:
                        state[c2] = accp.tile([P, 512], f32, tag="acc", name="acc")
                        if c2 == 0:
                            state["yo"] = ob.tile([P, C], fp16, tag="yo", name="yo")
                    nc.tensor.matmul(
                        state[c2][:],
                        attnT[kk][:, st * P:(st + 1) * P],
                        wo_sb[:, kk * C + c2 * 512:kk * C + (c2 + 1) * 512],
                        start=(kk == 0), stop=(kk == 1),
                    )
                    if kk == 1:
                        dst = state["yo"][:, c2 * 512:(c2 + 1) * 512]
                        if act_half and c2 == 1:
                            nc.scalar.activation(
                                dst, state[c2][:],
                                mybir.ActivationFunctionType.Copy)
                        else:
                            nc.vector.tensor_copy(dst, state[c2][:])
                        if c2 == 1:
                            nc.sync.dma_start(y[st * P:(st + 1) * P, :],
                                              state["yo"][:])
                return emit
            return [mk(c2, kk) for c2 in range(2) for kk in range(2)]

        def dummy_fillers(n):
            """cheap PE keep-alive matmuls for chunk tails whose real
            fillers run out before the ACT-serial diagonal exps finish —
            prevents the HAM down-clock right before the kernel tail."""
            def mk():
                def emit():
                    dps = accp.tile([P, 512], f32, tag="acc", name="acc")
                    nc.tensor.matmul(dps[:, 0:256], wmm[:, 0:P],
                                     wmm[:, 0:256], start=True, stop=True)
                return emit
            return [mk() for _ in range(n)]

        def chain(*gens):
            out = []
            for g in gens:
                out.extend(g)
            return out

        # ---------------- emission order ----------------------------------
        # t0 q/k chunks for S-half 0, rope, so exp can start early
        qk_chunk("q", 0, 0)
        qk_chunk("k", 0, 0)
        qk_chunk("q", 0, 1)
        qk_chunk("k", 0, 1)
        rope("q", 0, 0, 1024)
        rope("k", 0, 0, 1024)
        qk_chunk("q", 1, 0)
        qk_chunk("k", 1, 0)
        qk_chunk("q", 1, 1)
        qk_chunk("k", 1, 1)
        rope("q", 1, 0, 1024)
        rope("k", 1, 0, 1024)
        for st in range(4):
            v_tile(st)

        attention(0, 0, chain(v_fillers(4), v_fillers(5)))
        attention(0, 1, chain(v_fillers(6), v_fillers(7),
                              qk_fillers("q", 0, 2), qk_fillers("k", 0, 2)))
        attention(1, 1, chain(qk_fillers("q", 1, 2), qk_fillers("k", 1, 2),
                              v_fillers(8), v_fillers(9)))
        rope("q", 0, 1024, 512)
        rope("k", 0, 1024, 512)
        rope("q", 1, 1024, 512)
        rope("k", 1, 1024, 512)
        attention(0, 2, chain(v_fillers(10), v_fillers(11),
                              qk_fillers("q", 0, 3), qk_fillers("k", 0, 3)))
        attention(1, 2, chain(qk_fillers("q", 1, 3), qk_fillers("k", 1, 3),
                              v_fillers(12), v_fillers(13)))
        rope("q", 0, 1536, 512)
        rope("k", 0, 1536, 512)
        rope("q", 1, 1536, 512)
        rope("k", 1, 1536, 512)
        attention(0, 3, chain(v_fillers(14), v_fillers(15),
                              op_fillers(4), op_fillers(5),
                              dummy_fillers(10)))
        attention(1, 3, chain(op_fillers(6), op_fillers(7),
                              op_fillers(8), op_fillers(9),
                              op_fillers(10), op_fillers(11),
                              dummy_fillers(7)))
        attention(1, 0, chain(op_fillers(12, act_half=True),
                              op_fillers(13, act_half=True),
                              op_fillers(14, act_half=True),
                              op_fillers(15, act_half=True)))
        for st in range(4):
            out_proj(st, act_half=True)


def _host_inputs(x, w_qkv, q_bias, v_bias, w_out):
    """Build the 8 per-core input maps (SBUF-layout megatiles)."""
    half = D // 2
    # pair-split column permutation within each head's 64 cols
    perm64 = np.empty(D, dtype=np.int64)
    perm64[:half] = 2 * np.arange(half)
    perm64[half:] = 2 * np.arange(half) + 1

    dim_t = (TEMP ** (np.arange(half, dtype=np.float32) / half)).astype(np.float32)
    ang = (np.arange(S, dtype=np.float32)[None, :] / dim_t[:, None]).astype(np.float32)
    cos32 = np.cos(ang).astype(np.float32)      # [32, S]
    # psin = 32-block-swapped sign pattern [+s,-s,+s,-s]; host sends +sin
    # rows, the kernel replicates and negates blocks 1 and 3 on-chip
    sin32 = np.sin(ang).astype(np.float32)

    r = np.arange(P)
    tri01 = (r[None, :] >= r[:, None]).astype(np.float16)

    Wq = w_qkv[:, 0:C]
    Wk = w_qkv[:, C:2 * C]
    Wv = w_qkv[:, 2 * C:3 * C]

    def mega(w):            # [C, X] -> [128, CKT*X] k-major
        X = w.shape[1]
        return np.ascontiguousarray(
            w.reshape(CKT, P, X).transpose(1, 0, 2).reshape(P, CKT * X))

    in_maps = []
    for core in range(N_CORES):
        b, g = core // HPC, core % HPC
        h0 = HPC * g
        cols = np.concatenate(
            [64 * h + perm64 for h in range(h0, h0 + HPC)]
        )                                        # permuted q/k cols, len 256
        vcols = np.arange(64 * h0, 64 * h0 + 256)
        wv260 = np.zeros((C, 260), dtype=np.float32)
        vb260 = np.zeros((1, 260), dtype=np.float32)
        wvc = Wv[:, vcols]
        vbc = v_bias[vcols]
        for hh_ in range(4):
            wv260[:, 65 * hh_:65 * hh_ + 64] = wvc[:, 64 * hh_:64 * hh_ + 64]
            vb260[0, 65 * hh_:65 * hh_ + 64] = vbc[64 * hh_:64 * hh_ + 64]
            vb260[0, 65 * hh_ + 64] = 1.0
        wo2 = w_out[64 * h0:64 * h0 + 256, :]    # [256, 1024]
        wom = np.ascontiguousarray(
            wo2.reshape(2, P, C).transpose(1, 0, 2).reshape(P, 2 * C))
        in_maps.append({
            "xT": np.ascontiguousarray(x[b].T).astype(np.float16),
            "wqm": mega(Wq[:, cols]).astype(np.float16),
            "wkm": mega(Wk[:, cols]).astype(np.float16),
            "wvm": mega(wv260).astype(np.float16),
            "wom": wom.astype(np.float16),
            "qbt": np.ascontiguousarray(
                q_bias[cols].reshape(2, P).T).astype(np.float32),
            "vb": vb260,
            "cosT": cos32.astype(np.float16), "psinT": sin32.astype(np.float16),
            "tri01": tri01,
        })
    return in_maps


def kernel(x, w_qkv, q_bias, v_bias, w_out, _trace=False):
    global _NC
    if _NC is None:
        _NC = _build()
    in_maps = _host_inputs(
        np.asarray(x, np.float32), np.asarray(w_qkv, np.float32),
        np.asarray(q_bias, np.float32), np.asarray(v_bias, np.float32),
        np.asarray(w_out, np.float32),
    )
    res = run_bass_kernel_spmd(_NC, in_maps, list(range(N_CORES)), trace=_trace)
    out = np.empty((B, S, C), dtype=np.float32)
    for b in range(B):
        acc = res.results[HPC * b]["y"].astype(np.float32)
        for g in range(1, HPC):
            acc += res.results[HPC * b + g]["y"].astype(np.float32)
        out[b] = acc
    if _trace:
        kernel.last_exec_time_ns = res.exec_time_ns
    return out


# revision 35
# speedup vs baseline: 1.0172x; 1.0105x over previous
"""EnhanceSelfAttention (B=2, S=2048, C=1024, H=16, D=64) on 8 trn2 cores.

Sharding: core c -> batch b = c // 4, head group g = c % 4 (heads 4g..4g+3).
Each core computes its 4 heads end-to-end plus a partial output projection
(rows of w_out for its heads); host sums the 4 partials per batch.

Schedule (v3), built around two measured facts from the v1 trace:
  - the attention inner loop is ACT-paced (exp [128,1024] = ~1147ns vs
    ~815ns of PE work per k-step), so ACT streams exp and nothing else
    until the post-exp tail;
  - input DMA is descriptor-rate-bound (~35ns/descriptor), so weights are
    host-packed into SBUF-layout megatiles (4KB rows), x streams in k-pair
    chunks (2KB rows) across queues, and cos/psin ship as 32 distinct rows
    replicated on-chip.
Other changes vs v1:
  - causal mask applied by DVE multiply (tri01) on the exp output
    instead of two PE matmuls per diagonal tile;
  - q_bias folded into the psum drain (tensor_scalar_add), not a K=1
    matmul; psum->SBUF drains on DVE; ACT takes the post-exp out-proj
    copies only;
  - attention chunks emit interleaved "filler" matmuls (later QKV
    chunks, V tiles, out-proj) between k-steps to fill PE slack during
    ACT-paced stretches; chunk order (0,0),(0,1),(1,1),(0,2),(1,2),
    (0,3),(1,3),(1,0) with quarter-granular RoPE for S-chunks 2/3;
  - softmax denominators ride the PV matmul as a 65th V column of ones;
    normalization = fast-reciprocal + gpsimd partition broadcast.
"""

import sys

if "/opt/trn_rl_repo" not in sys.path:
    sys.path.insert(0, "/opt/trn_rl_repo")

import numpy as np

import concourse.bacc as bacc
import concourse.bass as bass
import concourse.tile as tile
from concourse import mybir
from concourse.bass_utils import run_bass_kernel_spmd

B, S, C = 2, 2048, 1024
H, D = 16, 64
TEMP = 1e4
N_CORES = 8
HPC = 4            # heads per core
P = 128
NQC = S // 512     # 4 q-chunks of 512
KT = S // P        # 16 k-tiles
CKT = C // P       # 8 contraction tiles for projections

f32 = mybir.dt.float32
fp16 = mybir.dt.float16

_NC = None


def _build():
    nc = bacc.Bacc("TRN2", target_bir_lowering=False, debug=False)

    xT = nc.dram_tensor("xT", [C, S], fp16, kind="ExternalInput").ap()
    wqm = nc.dram_tensor("wqm", [P, CKT * 256], fp16, kind="ExternalInput").ap()
    wkm = nc.dram_tensor("wkm", [P, CKT * 256], fp16, kind="ExternalInput").ap()
    wvm = nc.dram_tensor("wvm", [P, CKT * 260], fp16, kind="ExternalInput").ap()
    wom = nc.dram_tensor("wom", [P, 2 * C], fp16, kind="ExternalInput").ap()
    qbt = nc.dram_tensor("qbt", [P, 2], f32, kind="ExternalInput").ap()
    vb = nc.dram_tensor("vb", [1, 260], f32, kind="ExternalInput").ap()
    cosT = nc.dram_tensor("cosT", [32, S], fp16, kind="ExternalInput").ap()
    psinT = nc.dram_tensor("psinT", [32, S], fp16, kind="ExternalInput").ap()
    tri01 = nc.dram_tensor("tri01", [P, P], fp16, kind="ExternalInput").ap()
    y = nc.dram_tensor("y", [S, C], fp16, kind="ExternalOutput").ap()

    with tile.TileContext(nc) as tc:
        _body(nc, tc, xT, wqm, wkm, wvm, wom, qbt, vb, cosT, psinT, tri01, y)
    nc.compile()
    return nc


def _body(nc, tc, xT, wqm, wkm, wvm, wom, qbt, vb, cosT, psinT, tri01, y):
    from contextlib import ExitStack

    with ExitStack() as ctx:
        consts = ctx.enter_context(tc.tile_pool(name="consts", bufs=1))

        xm = consts.tile([P, CKT * S], fp16, tag="xm", name="xm")
        wq_sb = consts.tile([P, CKT * 256], fp16, tag="wq", name="wq")
        wk_sb = consts.tile([P, CKT * 256], fp16, tag="wk", name="wk")
        wv_sb = consts.tile([P, CKT * 260], fp16, tag="wv", name="wv")
        wo_sb = consts.tile([P, 2 * C], fp16, tag="wo", name="wo")
        cos_sb = consts.tile([P, S], fp16, tag="cos", name="cos")
        psin_sb = consts.tile([P, S], fp16, tag="psin", name="psin")
        tri_sb = consts.tile([P, P], fp16, tag="tri", name="tri")
        qb_sb = consts.tile([P, 2], f32, tag="qb", name="qb")
        vb_sb = consts.tile([P, 260], f32, tag="vb", name="vb")
        warm = consts.tile([1, 2], f32, tag="warm", name="warm")
        wmm = consts.tile([P, 512], fp16, tag="wmm", name="wmm")

        qrot = [consts.tile([P, S], fp16, tag=f"qrot{t}", name=f"qrot{t}") for t in range(2)]
        krot = [consts.tile([P, S], fp16, tag=f"krot{t}", name=f"krot{t}") for t in range(2)]
        v_sb = [consts.tile([P, 260], fp16, tag=f"v{st}", name=f"v{st}") for st in range(KT)]
        attnT = [consts.tile([P, S], fp16, tag=f"attnT{p_}", name=f"attnT{p_}") for p_ in range(2)]

        # ---- input DMA: big-descriptor megatiles on sync/scalar/gpsimd ----
        xsrc = xT.rearrange("(k p) s -> p k s", p=P)
        xdst = xm[:].rearrange("p (k s) -> p k s", s=S)
        h0 = slice(0, 1024)
        h1 = slice(1024, 2048)
        # Priority ordering: each queue's transfers complete serially, so
        # late-needed bulk (x h1, wo) parks behind the critical stream.
        nc.sync.dma_start(xdst[:, 0:2, h0], xsrc[:, 0:2, h0])
        nc.sync.dma_start(xdst[:, 2:4, h0], xsrc[:, 2:4, h0])
        nc.sync.dma_start(xdst[:, 0:6, h1], xsrc[:, 0:6, h1])
        nc.sync.dma_start(xdst[:, 6:8, h1], xsrc[:, 6:8, h1])
        nc.sync.dma_start(wo_sb[:], wom[:])
        # weights on scalar (4KB descriptors); cos/psin ship as 32 rows
        nc.scalar.dma_start(wq_sb[:], wqm[:])
        nc.scalar.dma_start(wk_sb[:], wkm[:])
        nc.scalar.dma_start(cos_sb[0:32, :], cosT[:])
        nc.scalar.dma_start(psin_sb[0:32, :], psinT[:])
        # gpsimd: trailing x h0 k-tiles + small inputs + wv
        nc.gpsimd.dma_start(xdst[:, 4:6, h0], xsrc[:, 4:6, h0])
        nc.gpsimd.dma_start(xdst[:, 6:8, h0], xsrc[:, 6:8, h0])
        nc.gpsimd.dma_start(qb_sb[:], qbt[:])
        nc.gpsimd.dma_start(tri_sb[:], tri01[:])
        nc.gpsimd.dma_start(
            vb_sb[:],
            bass.AP(tensor=vb.tensor, offset=vb.offset, ap=[[0, P], [1, 260]]),
        )
        nc.gpsimd.dma_start(wv_sb[:], wvm[:])
        # replicate the 32 cos/psin rows to the other 3 partition blocks
        # (SBUF-SBUF, no HBM traffic); psin blocks 1,3 are negated
        for blk in range(1, 4):
            nc.gpsimd.dma_start(cos_sb[32 * blk:32 * blk + 32, :], cos_sb[0:32, :])
            nc.gpsimd.dma_start(psin_sb[32 * blk:32 * blk + 32, :], psin_sb[0:32, :])
        nc.vector.tensor_scalar_mul(psin_sb[32:64, :], psin_sb[32:64, :], -1.0)
        nc.vector.tensor_scalar_mul(psin_sb[96:128, :], psin_sb[96:128, :], -1.0)

        # ACT exp-table warm + warmup matmul data
        nc.vector.memset(warm[:], 0.0)
        nc.scalar.activation(warm[:], warm[:], mybir.ActivationFunctionType.Exp)
        nc.vector.memset(wmm[:], 0.0)

        # ---- pools (coexist; 8 PSUM banks total) ----
        accp = ctx.enter_context(tc.tile_pool(name="accp", bufs=4, space="PSUM"))
        scp = ctx.enter_context(tc.tile_pool(name="scps", bufs=2, space="PSUM"))
        bcosp = ctx.enter_context(tc.tile_pool(name="bcosp", bufs=4))
        swp = ctx.enter_context(tc.tile_pool(name="qkswp", bufs=3))
        exp_pool = ctx.enter_context(tc.tile_pool(name="expool", bufs=8))
        rcp_pool = ctx.enter_context(tc.tile_pool(name="rcppool", bufs=4))
        rbp = ctx.enter_context(tc.tile_pool(name="rbp", bufs=4))
        ob = ctx.enter_context(tc.tile_pool(name="obuf", bufs=6))

        # PE warm-up: release the HAM clock gate during the DMA window
        wps = accp.tile([P, 512], f32, tag="acc", name="acc")
        for _ in range(8):
            nc.tensor.matmul(wps[:], wmm[:, 0:P], wmm[:], start=True, stop=True)

        # ---------------- QKV projection + fused RoPE ---------------------
        base_t = {}   # (kind, t, half) -> [128, 1024] fp16 base tile

        def qk_chunk(kind, t, sc):
            """one [128, 512] psum accumulation for S-chunk sc; drain on DVE
            with q_bias folded in for q."""
            wsb = wq_sb if kind == "q" else wk_sb
            ps = accp.tile([P, 512], f32, tag="acc", name="acc")
            for k in range(CKT):
                nc.tensor.matmul(
                    ps[:],
                    wsb[:, k * 256 + t * P:k * 256 + (t + 1) * P],
                    xm[:, k * S + sc * 512:k * S + (sc + 1) * 512],
                    start=(k == 0), stop=(k == CKT - 1),
                )
            if (kind, t, sc // 2) not in base_t:
                base_t[(kind, t, sc // 2)] = bcosp.tile(
                    [P, 1024], fp16, tag="bcos", name="bcos")
            base = base_t[(kind, t, sc // 2)]
            dst = base[:, (sc % 2) * 512:(sc % 2) * 512 + 512]
            if kind == "q":
                nc.vector.tensor_scalar_add(dst, ps[:], qb_sb[:, t:t + 1])
            else:
                nc.vector.tensor_copy(dst, ps[:])

        def rope(kind, t, lo, w):
            """rotate base cols [lo, lo+w) (absolute S cols) into q/krot."""
            rot = (qrot if kind == "q" else krot)[t]
            span = slice(lo, lo + w)
            half = lo // 1024
            base = base_t[(kind, t, half)]
            bs = slice(lo - half * 1024, lo - half * 1024 + w)
            sw = swp.tile([P, 1024], fp16, tag="sw", name="sw")
            for blk in range(4):
                sb_, db = 32 * (blk ^ 1), 32 * blk
                nc.gpsimd.dma_start(sw[db:db + 32, bs], base[sb_:sb_ + 32, bs])
            nc.vector.tensor_mul(base[:, bs], base[:, bs], cos_sb[:, span])
            # psin = 32-block-swapped sin = -sin, so subtract: rot =
            # base*cos + swap(base)*sin = base*cos - swap(base)*psin
            nc.vector.tensor_mul(sw[:, bs], sw[:, bs], psin_sb[:, span])
            nc.vector.tensor_sub(rot[:, span], base[:, bs], sw[:, bs])

        def v_tile(st):
            ps = accp.tile([P, 260], f32, tag="acc", name="acc")
            for k in range(CKT):
                nc.tensor.matmul(
                    ps[:],
                    xm[:, k * S + st * P:k * S + (st + 1) * P],
                    wv_sb[:, k * 260:(k + 1) * 260],
                    start=(k == 0), stop=(k == CKT - 1),
                )
            nc.vector.tensor_add(v_sb[st][:], ps[:], vb_sb[:])

        # ---------------- attention with interleaved fillers --------------
        def attention(pair, n, fillers=()):
            fill = iter(fillers)

            def pump(cnt):
                for _ in range(cnt):
                    f = next(fill, None)
                    if f is None:
                        return
                    f()

            pv_ps = [accp.tile([65, 512], f32, tag="acc", name="acc")
                     for hh in range(2)]
            klast = 4 * n + 3

            def scores_step(k):
                diag = (k // 4 == n)
                off = P * (k % 4) if diag else 0
                sc = scp.tile([P, 2 * 512], f32, tag="sc", name="sc")
                for hh in range(2):
                    nc.tensor.matmul(
                        sc[:, hh * 512 + off:(hh + 1) * 512],
                        krot[pair][64 * hh:64 * hh + 64, k * P:(k + 1) * P],
                        qrot[pair][64 * hh:64 * hh + 64,
                                   n * 512 + off:(n + 1) * 512],
                        start=True, stop=True,
                        tile_position=(64 * hh, 0),
                    )
                ex = exp_pool.tile([P, 2 * 512], fp16, tag="ex", name="ex")
                if diag:
                    exr = ex[:].rearrange("p (h c) -> p h c", h=2)[:, :, off:]
                    scr = sc[:].rearrange("p (h c) -> p h c", h=2)[:, :, off:]
                    nc.scalar.activation(
                        exr, scr, mybir.ActivationFunctionType.Exp,
                        scale=0.125,
                    )
                    # causal mask: zero the upper triangle of the 128-wide
                    # diagonal block (DVE, not PE)
                    for hh in range(2):
                        blk = slice(hh * 512 + off, hh * 512 + off + P)
                        nc.vector.tensor_mul(ex[:, blk], ex[:, blk], tri_sb[:])
                else:
                    nc.scalar.activation(
                        ex[:], sc[:], mybir.ActivationFunctionType.Exp,
                        scale=0.125,
                    )
                return ex, off

            def pv_step(k, ex, off):
                for hh in range(2):
                    h = 2 * pair + hh
                    nc.tensor.matmul(
                        pv_ps[hh][:, off:512],
                        v_sb[k][:, 65 * h:65 * h + 65],
                        ex[:, hh * 512 + off:(hh + 1) * 512],
                        start=(k == 0), stop=(k == klast),
                    )

            # software pipeline: emit scores(k+1) before PV(k); pump
            # fillers into the PE slack of each ACT-paced step
            prev = None
            for k in range(klast + 1):
                cur = (k, *scores_step(k))
                pump(1)
                if prev is not None:
                    pv_step(*prev)
                    pump(1)
                prev = cur
            pv_step(*prev)
            span = slice(n * 512, (n + 1) * 512)
            dc = rcp_pool.tile([1, 1024], f32, tag="dc", name="dc")
            for hh in range(2):
                nc.vector.tensor_copy(dc[:, hh * 512:hh * 512 + 512],
                                      pv_ps[hh][64:65, :])
            rc = rcp_pool.tile([1, 1024], f32, tag="rc", name="rc")
            nc.vector.reciprocal_approx_fast(rc[:], dc[:])
            for hh in range(2):
                rb = rbp.tile([64, 512], f32, tag="rb", name="rb")
                nc.gpsimd.partition_broadcast(rb[:], rc[:, hh * 512:hh * 512 + 512])
                nc.vector.tensor_mul(
                    attnT[pair][64 * hh:64 * hh + 64, span],
                    pv_ps[hh][0:64, :],
                    rb[:],
                )
            # drain leftover fillers
            for f in fill:
                f()

        def out_proj(st, act_half=False):
            yo = ob.tile([P, C], fp16, tag="yo", name="yo")
            for c2 in range(2):
                po = accp.tile([P, 512], f32, tag="acc", name="acc")
                for kk in range(2):
                    nc.tensor.matmul(
                        po[:],
                        attnT[kk][:, st * P:(st + 1) * P],
                        wo_sb[:, kk * C + c2 * 512:kk * C + (c2 + 1) * 512],
                        start=(kk == 0), stop=(kk == 1),
                    )
                dst = yo[:, c2 * 512:(c2 + 1) * 512]
                if act_half and c2 == 1:
                    nc.scalar.activation(dst, po[:],
                                         mybir.ActivationFunctionType.Copy)
                else:
                    nc.vector.tensor_copy(dst, po[:])
            nc.sync.dma_start(y[st * P:(st + 1) * P, :], yo[:])

        # ---- filler generators: one closure ~= one matmul of PE work ----
        def qk_fillers(kind, t, sc):
            wsb = wq_sb if kind == "q" else wk_sb
            state = {}

            def mk(k):
                def emit():
                    if k == 0:
                        state["ps"] = accp.tile([P, 512], f32, tag="acc", name="acc")
                    nc.tensor.matmul(
                        state["ps"][:],
                        wsb[:, k * 256 + t * P:k * 256 + (t + 1) * P],
                        xm[:, k * S + sc * 512:k * S + (sc + 1) * 512],
                        start=(k == 0), stop=(k == CKT - 1),
                    )
                    if k == CKT - 1:
                        if (kind, t, sc // 2) not in base_t:
                            base_t[(kind, t, sc // 2)] = bcosp.tile(
                                [P, 1024], fp16, tag="bcos", name="bcos")
                        base = base_t[(kind, t, sc // 2)]
                        dst = base[:, (sc % 2) * 512:(sc % 2) * 512 + 512]
                        if kind == "q":
                            nc.vector.tensor_scalar_add(dst, state["ps"][:],
                                                        qb_sb[:, t:t + 1])
                        else:
                            nc.vector.tensor_copy(dst, state["ps"][:])
                return emit
            return [mk(k) for k in range(CKT)]

        def v_fillers(st):
            state = {}

            def mk(k):
                def emit():
                    if k == 0:
                        state["ps"] = accp.tile([P, 260], f32, tag="acc", name="acc")
                    nc.tensor.matmul(
                        state["ps"][:],
                        xm[:, k * S + st * P:k * S + (st + 1) * P],
                        wv_sb[:, k * 260:(k + 1) * 260],
                        start=(k == 0), stop=(k == CKT - 1),
                    )
                    if k == CKT - 1:
                        nc.vector.tensor_add(v_sb[st][:], state["ps"][:], vb_sb[:])
                return emit
            # 8 matmuls of 260 cols ~ 4 filler slots worth; group by 2
            fs = [mk(k) for k in range(CKT)]
            return [lambda a=fs[i], b=fs[i + 1]: (a(), b())
                    for i in range(0, CKT, 2)]

        def op_fillers(st, act_half=False):
            state = {}

            def mk(c2, kk):
                def emit():
                    if kk == 0:
                        state[c2] = accp.tile([P, 512], f32, tag="acc", name="acc")
                        if c2 == 0:
                            state["yo"] = ob.tile([P, C], fp16, tag="yo", name="yo")
                    nc.tensor.matmul(
                        state[c2][:],
                        attnT[kk][:, st * P:(st + 1) * P],
                        wo_sb[:, kk * C + c2 * 512:kk * C + (c2 + 1) * 512],
                        start=(kk == 0), stop=(kk == 1),
                    )
                    if kk == 1:
                        dst = state["yo"][:, c2 * 512:(c2 + 1) * 512]
                        if act_half and c2 == 1:
                            nc.scalar.activation(
                                dst, state[c2][:],
                                mybir.ActivationFunctionType.Copy)
                        else:
                            nc.vector.tensor_copy(dst, state[c2][:])
                        if c2 == 1:
                            nc.sync.dma_start(y[st * P:(st + 1) * P, :],
                                              state["yo"][:])
                return emit
            return [mk(c2, kk) for c2 in range(2) for kk in range(2)]

        def chain(*gens):
            out = []
            for g in gens:
                out.extend(g)
            return out

        # ---------------- emission order ----------------------------------
        # t0 q/k chunks for S-half 0, rope, so exp can start early
        qk_chunk("q", 0, 0)
        qk_chunk("k", 0, 0)
        qk_chunk("q", 0, 1)
        qk_chunk("k", 0, 1)
        rope("q", 0, 0, 1024)
        rope("k", 0, 0, 1024)
        qk_chunk("q", 1, 0)
        qk_chunk("k", 1, 0)
        qk_chunk("q", 1, 1)
        qk_chunk("k", 1, 1)
        rope("q", 1, 0, 1024)
        rope("k", 1, 0, 1024)
        for st in range(4):
            v_tile(st)

        attention(0, 0, chain(v_fillers(4), v_fillers(5)))
        attention(0, 1, chain(v_fillers(6), v_fillers(7),
                              qk_fillers("q", 0, 2), qk_fillers("k", 0, 2)))
        attention(1, 1, chain(qk_fillers("q", 1, 2), qk_fillers("k", 1, 2),
                              v_fillers(8), v_fillers(9)))
        rope("q", 0, 1024, 512)
        rope("k", 0, 1024, 512)
        attention(0, 2, chain(v_fillers(10), v_fillers(11),
                              qk_fillers("q", 0, 3), qk_fillers("k", 0, 3)))
        rope("q", 1, 1024, 512)
        rope("k", 1, 1024, 512)
        attention(1, 2, chain(qk_fillers("q", 1, 3), qk_fillers("k", 1, 3),
                              v_fillers(12), v_fillers(13)))
        rope("q", 0, 1536, 512)
        rope("k", 0, 1536, 512)
        attention(0, 3, chain(v_fillers(14), v_fillers(15),
                              op_fillers(4), op_fillers(5)))
        rope("q", 1, 1536, 512)
        rope("k", 1, 1536, 512)
        attention(1, 3, chain(op_fillers(6), op_fillers(7),
                              op_fillers(8), op_fillers(9),
                              op_fillers(10), op_fillers(11)))
        attention(1, 0, chain(op_fillers(12, act_half=True),
                              op_fillers(13, act_half=True),
                              op_fillers(14, act_half=True),
                              op_fillers(15, act_half=True)))
        for st in range(4):
            out_proj(st, act_half=True)


def _host_inputs(x, w_qkv, q_bias, v_bias, w_out):
    """Build the 8 per-core input maps (SBUF-layout megatiles)."""
    half = D // 2
    # pair-split column permutation within each head's 64 cols
    perm64 = np.empty(D, dtype=np.int64)
    perm64[:half] = 2 * np.arange(half)
    perm64[half:] = 2 * np.arange(half) + 1

    dim_t = (TEMP ** (np.arange(half, dtype=np.float32) / half)).astype(np.float32)
    ang = (np.arange(S, dtype=np.float32)[None, :] / dim_t[:, None]).astype(np.float32)
    cos32 = np.cos(ang).astype(np.float32)      # [32, S]
    # psin = 32-block-swapped sign pattern [+s,-s,+s,-s]; host sends +sin
    # rows, the kernel replicates and negates blocks 1 and 3 on-chip
    sin32 = np.sin(ang).astype(np.float32)

    r = np.arange(P)
    tri01 = (r[None, :] >= r[:, None]).astype(np.float16)

    Wq = w_qkv[:, 0:C]
    Wk = w_qkv[:, C:2 * C]
    Wv = w_qkv[:, 2 * C:3 * C]

    def mega(w):            # [C, X] -> [128, CKT*X] k-major
        X = w.shape[1]
        return np.ascontiguousarray(
            w.reshape(CKT, P, X).transpose(1, 0, 2).reshape(P, CKT * X))

    in_maps = []
    for core in range(N_CORES):
        b, g = core // HPC, core % HPC
        h0 = HPC * g
        cols = np.concatenate(
            [64 * h + perm64 for h in range(h0, h0 + HPC)]
        )                                        # permuted q/k cols, len 256
        vcols = np.arange(64 * h0, 64 * h0 + 256)
        wv260 = np.zeros((C, 260), dtype=np.float32)
        vb260 = np.zeros((1, 260), dtype=np.float32)
        wvc = Wv[:, vcols]
        vbc = v_bias[vcols]
        for hh_ in range(4):
            wv260[:, 65 * hh_:65 * hh_ + 64] = wvc[:, 64 * hh_:64 * hh_ + 64]
            vb260[0, 65 * hh_:65 * hh_ + 64] = vbc[64 * hh_:64 * hh_ + 64]
            vb260[0, 65 * hh_ + 64] = 1.0
        wo2 = w_out[64 * h0:64 * h0 + 256, :]    # [256, 1024]
        wom = np.ascontiguousarray(
            wo2.reshape(2, P, C).transpose(1, 0, 2).reshape(P, 2 * C))
        in_maps.append({
            "xT": np.ascontiguousarray(x[b].T).astype(np.float16),
            "wqm": mega(Wq[:, cols]).astype(np.float16),
            "wkm": mega(Wk[:, cols]).astype(np.float16),
            "wvm": mega(wv260).astype(np.float16),
            "wom": wom.astype(np.float16),
            "qbt": np.ascontiguousarray(
                q_bias[cols].reshape(2, P).T).astype(np.float32),
            "vb": vb260,
            "cosT": cos32.astype(np.float16), "psinT": sin32.astype(np.float16),
            "tri01": tri01,
        })
    return in_maps


def kernel(x, w_qkv, q_bias, v_bias, w_out, _trace=False):
    global _NC
    if _NC is None:
        _NC = _build()
    in_maps = _host_inputs(
        np.asarray(x, np.float32), np.asarray(w_qkv, np.float32),
        np.asarray(q_bias, np.float32), np.asarray(v_bias, np.float32),
        np.asarray(w_out, np.float32),
    )
    res = run_bass_kernel_spmd(_NC, in_maps, list(range(N_CORES)), trace=_trace)
    out = np.empty((B, S, C), dtype=np.float32)
    for b in range(B):
        acc = res.results[HPC * b]["y"].astype(np.float32)
        for g in range(1, HPC):
            acc += res.results[HPC * b + g]["y"].astype(np.float32)
        out[b] = acc
    if _trace:
        kernel.last_exec_time_ns = res.exec_time_ns
    return out
